# revision 29
# baseline (speedup 1.0000x reference)
"""Bass/Trainium2 kernel for cubic B-spline encoding (nn_BsplineEncoding).

Reference (per point p, per input dim d of 3):
    xs  = clip((x+1)*30.5, 0, 61-1e-6);  i = floor(xs);  u = xs - i
    out row = concat over d of [x_d, feat_d(64)] where feat_d[i..i+3] =
    cubic B-spline coefficients of u, rest 0.

Dense identity: feat_d[k] = b3(s) with s = xs + 3 - k (cardinal cubic
B-spline on [0,4]).  With t = min(s, 4-s), w = relu(t), v = relu(t-1):
    b3(s) = (w^3 - 4 v^3) / 6        (exact, incl. zeros outside [0,4])

Work is done in PAIRS of 1024-point groups (J=8 points per partition per
group); per pair:
  - PE: one 128x128 bf16 transpose brings both groups' x into lhsT
    orientation; per group, 3 bf16 matmuls compute
    s[p,(j,d,k)] = 30.5*x + (33.5-k) into PSUM f32. x enters as an exact
    bf16 hi/lo split (24+24 rows) plus a ones-row carrying the bias, so
    all matmul weights are bf16-exact.
  - ScalarE: per group, a = Abs(lam*s - 2*lam) fused PSUM->SBUF fp16
    drain, lam = 6^(-1/3) (lam^3 = 1/6); plus the transpose drain.
  - VectorE: one fp16 4x-mode tensor_scalar m2 = (a sub 2*lam) min 0
    (= -lam*w) over both groups, then ONE custom-DVE instruction
    (registered at import):
        m1 = min(m2 + lam, 0)  (= -lam*v);  out = 4*m1^3 - m2^3
    which equals (w^3 - 4v^3)/6 exactly, written fp16 strided into the
    out tile.
  - Pool: x passthrough column, pad memset, bf16 hi/lo split of x.
  - Output rows are stored 198 wide as [feat_d(64), x_d, pad] * 3 so all
    strided writes are 4B-aligned; the host permutes columns back to the
    reference layout [x_d, feat_d(64)] * 3 and upcasts fp16 -> f32.
"""

import math
import os
import sys
from contextlib import ExitStack

import numpy as np

for _p in ("/opt/trn_rl_repo", "/root/.axon_site/_ro/trn_rl_repo"):
    if os.path.isdir(_p) and _p not in sys.path:
        sys.path.insert(0, _p)

import concourse.bass as bass  # noqa: E402
import concourse.tile as tile  # noqa: E402
from concourse import bacc, mybir  # noqa: E402
from concourse import bass_utils  # noqa: E402
from concourse import dve_ops as _dve_ops_mod  # noqa: E402
from concourse.dve_spec import (  # noqa: E402
    Spec, Src0, C0, C1, Zero, minn, sq, lower as _dve_lower, _has_src1,
)
from concourse.dve_uop import DveOpSpec  # noqa: E402

F32 = mybir.dt.float32
F16 = mybir.dt.float16
BF16 = mybir.dt.bfloat16

N_CORES = 8
D = 3
K = 64
ROW = D * (1 + K)          # 195 f32 per output row (reference layout)
CW = 1 + K + 1             # 66: on-device per-dim block [feat(64), x, pad]
ROWP = D * CW              # 198 fp16 per on-device row
J = 8                      # points per partition per group
GROUP = 128 * J            # 1024 points per group
PAIR = 2 * GROUP           # transpose granularity (two groups share one)
BUNDLE = 4 * GROUP         # DVE/out ops cover four groups at a time
FEAT = J * D * K           # 1536 s-values per partition per group
SCALE = (K - 3) / 2.0      # 30.5
MM_CHUNKS = ((0, 512), (512, 1024), (1024, 1536))
MAX_SG = 16                # groups per input-DMA supergroup (even)
LAM = 6.0 ** (-1.0 / 3.0)  # lam^3 = 1/6
JD = J * D                 # 24
CG = 64                    # xhl cols per group: 24 hi | 24 lo | 1 one | pad
TS_DVE = 1920              # of the 2*FEAT=3072 clamp cols, DVE does this many


def _bspline_ref(in0, in1, s0, s1, imm2):
    m2 = np.asarray(in0, np.float32)
    m1 = np.minimum(m2 + s0, 0)
    return (s1 * m1 ** 3 - m2 ** 3).astype(np.float32)


def _register_bspline_op():
    name = "BSPLINE_TAIL_ANT"
    for op in _dve_ops_mod.OPS:
        if op.name == name:
            return op
    m1 = minn(Src0 + C0, Zero)
    m2c = sq(Src0) * Src0
    m1c = sq(m1) * m1
    body = m1c * C1 - m2c
    spec = Spec(body=body, reference=_bspline_ref)
    shas = {}
    for ver in ("v3", "v4"):
        uops = _dve_lower(spec, ver=ver)
        shas[ver] = DveOpSpec(
            name=name, uops=uops, rd1_en=_has_src1(spec)).sha(ver)
    op = _dve_ops_mod.DveOp(name, spec, subdim=False, uops_sha=shas)
    _dve_ops_mod.OPS.append(op)
    row = _dve_ops_mod._CUSTOM_DVE_ROW_BASE + len(_dve_ops_mod.OPS) - 1
    assert row < 0x20, f"custom DVE row {row} overflows 5-bit field"
    _dve_ops_mod._SUB_OPCODE_FOR_NAME[name] = row
    return op


BSPLINE_OP = _register_bspline_op()





def _host_consts():
    ident = np.eye(128, dtype=np.float32)  # cast to bf16 at shard time
    # E2 [128, FEAT], identical 49-row blocks at partition 0 and 64 (one per
    # pair half, matching each lhsT slice's base partition): rows +0..23 =
    # x_hi weight 30.5 in that (j,d)'s K-block, rows +24..47 = x_lo weight
    # 30.5, row +48 = bias (33.5 - k). All values bf16-exact.
    E2 = np.zeros((128, FEAT), dtype=np.float32)
    for base in (0, CG):
        for m in range(JD):
            E2[base + m, m * K:(m + 1) * K] = SCALE
            E2[base + JD + m, m * K:(m + 1) * K] = SCALE
            E2[base + 2 * JD, m * K:(m + 1) * K] = (SCALE + 3.0) - np.arange(K)
    return ident, E2


def _split_supergroups(n_groups):
    assert n_groups % 4 == 0
    sizes = []
    left = n_groups
    while left > 0:
        g = min(MAX_SG, left)
        rem = left - g
        if rem and rem % 4:
            g -= 4 - rem % 4
        sizes.append(g)
        left -= g
    assert all(s % 4 == 0 for s in sizes)
    return sizes


def build_program(npad):
    """Per-core Bass program for npad (multiple of BUNDLE) points."""
    assert npad % BUNDLE == 0
    n_groups = npad // GROUP
    nc = bacc.Bacc("TRN2", target_bir_lowering=False, debug=False,
                   num_devices=N_CORES)
    x_d = nc.dram_tensor("x", [npad, D], F32, kind="ExternalInput").ap()
    out_d = nc.dram_tensor("out", [npad, ROWP], F16,
                           kind="ExternalOutput").ap()
    ident_d = nc.dram_tensor("ident", [128, 128], BF16,
                             kind="ExternalInput").ap()
    e2_d = nc.dram_tensor("e2", [128, FEAT], BF16,
                          kind="ExternalInput").ap()

    AL = mybir.AluOpType
    ACTF = mybir.ActivationFunctionType

    with tile.TileContext(nc) as tc, ExitStack() as ctx:
        cpool = ctx.enter_context(tc.tile_pool(name="const", bufs=1))
        ident_t = cpool.tile([128, 128], BF16, tag="ident")
        nc.sync.dma_start(ident_t[:], ident_d[:])
        e2_t = cpool.tile([128, FEAT], BF16, tag="e2")
        nc.sync.dma_start(e2_t[:], e2_d[:])
        b_act = cpool.tile([128, 1], F32, tag="b_act")
        nc.vector.memset(b_act[:], -2.0 * LAM)
        # dummy activation: pulls the ~1.3us ACT_TABLE_LOAD off the first
        # drain's critical path (overlaps the const/input DMAs instead)
        warm = cpool.tile([128, 1], F16, tag="warm")
        nc.scalar.activation(warm[:], b_act[:], ACTF.Abs, bias=b_act[:],
                             scale=1.0)

        xin_p = ctx.enter_context(tc.tile_pool(name="xin", bufs=2))
        xhl_p = ctx.enter_context(tc.tile_pool(name="xhl", bufs=2))
        xT_p = ctx.enter_context(tc.tile_pool(name="xT", bufs=3))
        scr_p = ctx.enter_context(tc.tile_pool(name="scr", bufs=2))
        out_p = ctx.enter_context(tc.tile_pool(name="out", bufs=3))
        psT_p = ctx.enter_context(tc.tile_pool(name="psT", bufs=2,
                                               space="PSUM"))
        psS_p = ctx.enter_context(tc.tile_pool(name="psS", bufs=2,
                                               space="PSUM"))

        sgs = _split_supergroups(n_groups)
        starts = [sum(sgs[:i]) for i in range(len(sgs))]

        def emit_sg_prep(G, g0, first=False):
            """Input DMA + bf16 hi/lo split for one supergroup; issued one
            supergroup early so the SP queue dispatches the prefetch before
            the current supergroup's output DMAs. The first supergroup's
            lo-split runs on the (fast) Vector engine because it sits in the
            startup latency chain; later ones run on the idle Pool engine,
            off Vector's steady-state critical path."""
            b0 = g0 * GROUP
            x_sl = x_d[b0:b0 + GROUP * G, :].rearrange(
                "(p k) d -> p (k d)", p=128)
            xin = xin_p.tile([128, G * JD], F32, tag="xin", name="xin")
            nc.sync.dma_start(xin[:], x_sl)
            xhl = xhl_p.tile([128, G * CG], BF16, tag="xhl", name="xhl")
            xhl_g = xhl[:].rearrange("p (g c) -> p g c", g=G)
            xin_g = xin[:].rearrange("p (g m) -> p g m", g=G)
            nc.gpsimd.tensor_copy(xhl_g[:, :, 0:JD], xin_g)
            lo_eng = nc.vector if first else nc.gpsimd
            lo_eng.tensor_tensor(
                xhl_g[:, :, JD:2 * JD], xin_g, xhl_g[:, :, 0:JD],
                AL.subtract)
            nc.gpsimd.memset(xhl_g[:, :, 2 * JD:2 * JD + 1], 1.0)
            return xin, xhl

        pend = emit_sg_prep(sgs[0], starts[0], first=True)
        for si, G in enumerate(sgs):
            g0 = starts[si]
            b0 = g0 * GROUP
            xin, xhl = pend
            if si + 1 < len(sgs):
                pend = emit_sg_prep(sgs[si + 1], starts[si + 1])
            # per-pair DMA view: [pair, 128, 2*J rows of ROWP]
            out_sl = out_d[b0:b0 + GROUP * G, :].rearrange(
                "(p q j) f -> q p (j f)", p=128, j=2 * J)
            for q in range(G // 2):
                gA = 2 * q
                # one transpose covers both groups of the pair
                psT = psT_p.tile([128, 128], BF16, tag="psT", name="psT")
                nc.tensor.transpose(
                    psT[:], xhl[:, gA * CG:(gA + 2) * CG], ident_t[:])
                xT = xT_p.tile([128, 128], BF16, tag="xT", name="xT")
                nc.scalar.copy(xT[:], psT[:])

                a = scr_p.tile([128, 2 * FEAT], F16, tag="a", name="a")
                for h in range(2):
                    psS = psS_p.tile([128, FEAT], F32, tag="psS",
                                     name="psS")
                    hb = CG * h
                    lhsT = xT[hb:hb + 2 * JD + 1, :]
                    for c0, c1 in MM_CHUNKS:
                        nc.tensor.matmul(
                            psS[:, c0:c1], lhsT,
                            e2_t[hb:hb + 2 * JD + 1, c0:c1],
                            start=True, stop=True)
                    nc.scalar.activation(a[:, h * FEAT:(h + 1) * FEAT],
                                         psS[:], ACTF.Abs, bias=b_act[:],
                                         scale=LAM)

                m2 = scr_p.tile([128, 2 * FEAT], F16, tag="m2", name="m2")
                nc.vector.tensor_scalar(m2[:], a[:], 2.0 * LAM, 0.0,
                                        AL.subtract, AL.min)

                out_t = out_p.tile([128, 2 * J * ROWP], F16, tag="out",
                                   name="out_t")
                ov = out_t[:].rearrange("p (m c) -> p m c", c=CW)
                m2v = m2[:].rearrange("p (m k) -> p m k", k=K)
                nc.vector._custom_dve(
                    BSPLINE_OP, out=ov[:, :, 0:K], in0=m2v,
                    s0=LAM, s1=4.0)
                xsrc = xin[:, gA * JD:(gA + 2) * JD].rearrange(
                    "p (m o) -> p m o", o=1)
                nc.gpsimd.tensor_copy(ov[:, :, K:K + 1], xsrc)
                nc.gpsimd.memset(ov[:, :, K + 1:K + 2], 0.0)
                nc.sync.dma_start(out_sl[q], out_t[:])


    nc.compile()
    _dedup_ldweights(nc)
    return nc


def _ldw_key(i):
    ap = i.ins[0]
    return (ap.memref, ap.offset, str(ap.ap), str(ap.dtype),
            str(i.perf_mode), str(i.is_transpose))


def _dedup_ldweights(nc):
    """Drop InstLdweights that reload the exact weights already resident in
    the PE array (the 3 chunk matmuls per group share one stationary
    operand). The engine keeps weights across matmuls; walrus codegen keys
    LDW emission off the explicit instruction stream."""
    for f in nc.m.functions:
        for b in f.blocks:
            insts = b.instructions
            keep = []
            last = None
            removed = set()
            for i in insts:
                nm = type(i).__name__
                if nm == "InstLdweights":
                    k = _ldw_key(i)
                    if last is not None and k == last:
                        removed.add(i.name)
                        continue
                    last = k
                keep.append(i)
            if removed:
                for i in keep:
                    try:
                        i.remap_dependency_names(
                            {n: None for n in removed})
                    except Exception:
                        pass
                b.instructions = keep


_CACHE = {}


def _get_program(npad):
    if npad not in _CACHE:
        _CACHE[npad] = build_program(npad)
    return _CACHE[npad]


def _postprocess(o, nsh):
    """[npad, 198] fp16 device layout -> [nsh, 195] f32 reference layout."""
    o = o[:nsh]
    out = np.empty((nsh, ROW), dtype=np.float32)
    for d in range(D):
        blk = o[:, d * CW:d * CW + K + 1].astype(np.float32)
        out[:, d * (K + 1)] = blk[:, K]
        out[:, d * (K + 1) + 1:(d + 1) * (K + 1)] = blk[:, 0:K]
    return out


def run_sharded(x, trace=False):
    """x: [N, 3] f32, N divisible by N_CORES. Returns ([N,195] f32, res)."""
    n = x.shape[0]
    assert n % N_CORES == 0
    nsh = n // N_CORES
    npad = int(math.ceil(nsh / BUNDLE)) * BUNDLE
    nc = _get_program(npad)
    ident, E2 = _host_consts()
    _bf16 = mybir.dt.np(BF16)
    ident_b = ident.astype(_bf16)
    e2_b = E2.astype(_bf16)
    in_maps = []
    for i in range(N_CORES):
        sh = np.asarray(x[i * nsh:(i + 1) * nsh], dtype=np.float32)
        if npad != nsh:
            sh = np.concatenate(
                [sh, np.zeros((npad - nsh, D), np.float32)], axis=0)
        in_maps.append({
            "x": np.ascontiguousarray(sh),
            "ident": ident_b, "e2": e2_b,
        })
    res = bass_utils.run_bass_kernel_spmd(
        nc, in_maps, core_ids=list(range(N_CORES)), trace=trace)
    outs = []
    for i in range(N_CORES):
        outs.append(_postprocess(np.asarray(res.results[i]["out"]), nsh))
    return np.concatenate(outs, axis=0), res


def kernel(x):
    x = np.asarray(x, dtype=np.float32)
    out, _ = run_sharded(x, trace=False)
    return out
